# revision 11
# baseline (speedup 1.0000x reference)
"""MoE gate routing kernel for Trainium2 (8 NeuronCores, data-parallel over tokens).

Computes, for x[8192,7168], weight[256,7168], bias[256]:
    scores = sigmoid(x @ weight.T + bias)            # [N, 256]
    group top-2 sums over 8 groups of 32 -> pick best group
    top-8 experts within best group (global indices), weights = renormalized
    sigmoid scores * 2.5
Returns (w [8192,8] f32, idx [8192,8] i32).

Strategy: shard tokens 8-way (1024/core). Host pre-packs x and weight into the
exact SBUF tile layouts so every DMA descriptor line is a long contiguous run
(28KB/partition vs 1KB for a naive transposed layout). Weights load as 4
K-groups so the first matmuls only wait on ~5.5MB of DMA instead of ~15MB.
Outputs accumulate in SBUF and leave as one DMA per tensor at the end, keeping
the SP DMA stream free of mid-loop waits on the vector pipeline. Matmul runs
as float32r (full-rate fp32). Bias is preloaded into PSUM via a K=1
ones-matmul. Sigmoid on ScalarE; group-top2 / top-8 / renorm on VectorE via
tensor_reduce, match_replace, max/max_index.
"""

import sys

sys.path.insert(0, "/opt/trn_rl_repo")

from concurrent.futures import ThreadPoolExecutor

import numpy as np

import concourse.bass as bass
from concourse import bacc
import concourse.mybir as mybir
from concourse.bass_utils import run_bass_kernel_spmd
from concourse.tile import TileContext

N_CORES = 8
N_TOK = 8192
TOK_PC = N_TOK // N_CORES  # 1024 tokens per core
D = 7168
E = 256
G = 8  # groups
EPG = E // G  # 32 experts per group
TOPK = 8
ROUTE_SCALE = 2.5
KC = D // 128  # 56 k-chunks
XBUF_T = 128  # tokens per x DMA buffer
NBUF = TOK_PC // XBUF_T  # 8 buffers == token tiles
NQ = 4  # weight K-groups
KCQ = KC // NQ  # 14 k-chunks per group

f32 = mybir.dt.float32
f32r = mybir.dt.float32r
i32 = mybir.dt.int32
u32 = mybir.dt.uint32
AX = mybir.AxisListType
OP = mybir.AluOpType
ACTF = mybir.ActivationFunctionType

_cache = {}
LAST_RESULTS = None


def _build():
    nc = bacc.Bacc(None, target_bir_lowering=False)

    # host-packed: x5[tb, q, p, c, n] = x[tok0 + tb*XBUF_T + n, (q*KCQ+c)*128 + p]
    x5 = nc.declare_dram_parameter(
        "x5", [NBUF, NQ, 128, KCQ, XBUF_T], f32, isOutput=False
    )
    # w5[q, p, c, e] = weight[e, (q*KCQ+c)*128 + p]
    w5 = nc.declare_dram_parameter("w5", [NQ, 128, KCQ, E], f32, isOutput=False)
    bias = nc.declare_dram_parameter("bias", [1, E], f32, isOutput=False)
    # outputs tile-major: w_out[i, p, k] = w for token i*128+p
    w_out = nc.declare_dram_parameter("w_out", [NBUF, 128, TOPK], f32, isOutput=True)
    idx_out = nc.declare_dram_parameter("idx_out", [NBUF, 128, TOPK], i32, isOutput=True)

    with TileContext(nc) as tc:
        with (
            tc.tile_pool(name="const", bufs=1) as cpool,
            tc.tile_pool(name="xbuf", bufs=3) as xpool,
            tc.tile_pool(name="sb", bufs=3) as spool,
            tc.tile_pool(name="small", bufs=3) as mpool,
            tc.tile_pool(name="out", bufs=1) as opool,
            tc.tile_pool(name="psum", bufs=6, space="PSUM") as ppool,
        ):
            bias_sb = cpool.tile([1, E], f32)
            nc.sync.dma_start(out=bias_sb, in_=bias[:, :])
            ones_sb = cpool.tile([1, 128], f32)
            nc.vector.memset(ones_sb, 1.0)

            # weight K-groups: separate tiles so early matmuls gate per-group
            wt = [
                cpool.tile([128, KCQ, E], f32r, name=f"wt{q}") for q in range(NQ)
            ]
            nc.sync.dma_start(out=wt[0], in_=w5[0].bitcast(f32r))
            nc.sync.dma_start(out=wt[1], in_=w5[1].bitcast(f32r))

            # output accumulation tiles (free-dim slice per token tile)
            w_acc = opool.tile([128, NBUF, TOPK], f32)
            i_acc = opool.tile([128, NBUF, TOPK], u32)

            for tb in range(NBUF):
                # quarter-split buffer DMAs: matmuls gate per quarter, so PE
                # idle gaps stay under the ~3.4us HAM re-throttle window
                xq = []
                for q in range(NQ):
                    xqt = xpool.tile([128, KCQ, XBUF_T], f32r, tag=f"xq{q}")
                    nc.sync.dma_start(out=xqt, in_=x5[tb, q].bitcast(f32r))
                    xq.append(xqt)
                if tb == 0:
                    nc.sync.dma_start(out=wt[2], in_=w5[2].bitcast(f32r))
                    nc.sync.dma_start(out=wt[3], in_=w5[3].bitcast(f32r))

                ps = ppool.tile([128, E], f32, tag="ps")
                # bias preload: ps[t, e] = 1 * bias[e]
                nc.tensor.matmul(
                    out=ps, lhsT=ones_sb, rhs=bias_sb, start=True, stop=False
                )
                for c in range(KC):
                    q, cq = divmod(c, KCQ)
                    nc.tensor.matmul(
                        out=ps,
                        lhsT=xq[q][:, cq, :],
                        rhs=wt[q][:, cq, :],
                        start=False,
                        stop=(c == KC - 1),
                    )

                sig = spool.tile([128, G, EPG], f32, tag="sig")
                nc.scalar.activation(
                    out=sig.rearrange("p g e -> p (g e)"), in_=ps, func=ACTF.Sigmoid
                )
                sig_flat = sig.rearrange("p g e -> p (g e)")

                # group top-2 sum
                m1 = mpool.tile([128, G], f32, tag="m1")
                nc.vector.tensor_reduce(out=m1, in_=sig, axis=AX.X, op=OP.max)
                scr = spool.tile([128, G, EPG], f32, tag="scr")
                nc.vector.match_replace(
                    out=scr.rearrange("p g e -> p (g e)"),
                    in_to_replace=m1,
                    in_values=sig_flat,
                    imm_value=-1e30,
                )
                gs = mpool.tile([128, G], f32, tag="gs")
                nc.vector.tensor_reduce(out=gs, in_=scr, axis=AX.X, op=OP.max)
                nc.vector.tensor_add(gs, gs, m1)  # m1 + m2

                # one-hot of best group -> multiplicative mask
                gmax = mpool.tile([128, 1], f32, tag="gmax")
                nc.vector.tensor_reduce(out=gmax, in_=gs, axis=AX.X, op=OP.max)
                eq = mpool.tile([128, G], f32, tag="eq")
                nc.vector.tensor_scalar(
                    eq, gs, gmax, None, op0=OP.is_ge
                )
                # masked scores: kept group unchanged (x1.0), others -> 0.0
                masked = spool.tile([128, G, EPG], f32, tag="masked")
                for g in range(G):
                    nc.vector.tensor_scalar(
                        masked[:, g, :],
                        sig[:, g, :],
                        eq[:, g : g + 1],
                        None,
                        op0=OP.mult,
                    )
                masked_flat = masked.rearrange("p g e -> p (g e)")

                vals8 = mpool.tile([128, TOPK], f32, tag="vals8")
                nc.vector.max(out=vals8, in_=masked_flat)
                nc.vector.max_index(
                    out=i_acc[:, tb, :], in_max=vals8, in_values=masked_flat
                )

                ssum = mpool.tile([128, 1], f32, tag="ssum")
                nc.vector.tensor_reduce(out=ssum, in_=vals8, axis=AX.X, op=OP.add)
                rcp = mpool.tile([128, 1], f32, tag="rcp")
                nc.vector.reciprocal(out=rcp, in_=ssum)
                nc.vector.tensor_scalar(
                    w_acc[:, tb, :], vals8, rcp, ROUTE_SCALE, op0=OP.mult, op1=OP.mult
                )

            nc.sync.dma_start(out=w_out.rearrange("i p k -> p i k"), in_=w_acc)
            nc.sync.dma_start(
                out=idx_out.rearrange("i p k -> p i k"), in_=i_acc.bitcast(i32)
            )
    nc.compile()
    return nc


def _pack_x(x, c):
    xs = x[c * TOK_PC : (c + 1) * TOK_PC]  # [1024, 7168]
    # x5[tb, q, p, cq, n] = xs[tb*XBUF_T + n, (q*KCQ+cq)*128 + p]
    v = xs.reshape(NBUF, XBUF_T, NQ, KCQ, 128)  # [tb, n, q, cq, p]
    return np.ascontiguousarray(v.transpose(0, 2, 4, 3, 1))


def kernel(x, weight, bias):
    global LAST_RESULTS
    x = np.ascontiguousarray(x, dtype=np.float32)
    weight = np.ascontiguousarray(weight, dtype=np.float32)
    bias = np.ascontiguousarray(bias, dtype=np.float32).reshape(1, E)

    if "nc" not in _cache:
        _cache["nc"] = _build()
    nc = _cache["nc"]

    # w5[q, p, cq, e] = weight[e, (q*KCQ+cq)*128 + p]
    w5 = np.ascontiguousarray(
        weight.reshape(E, NQ, KCQ, 128).transpose(1, 3, 2, 0)
    )

    with ThreadPoolExecutor(N_CORES) as ex:
        x_shards = list(ex.map(lambda c: _pack_x(x, c), range(N_CORES)))

    in_maps = [
        {"x5": x_shards[c], "w5": w5, "bias": bias} for c in range(N_CORES)
    ]
    LAST_RESULTS = run_bass_kernel_spmd(nc, in_maps, list(range(N_CORES)))
    res = LAST_RESULTS.results
    w = np.concatenate(
        [res[c]["w_out"].reshape(TOK_PC, TOPK) for c in range(N_CORES)], axis=0
    )
    idx = np.concatenate(
        [res[c]["idx_out"].reshape(TOK_PC, TOPK) for c in range(N_CORES)], axis=0
    )
    return w, idx.astype(np.int32)


# revision 15
# speedup vs baseline: 1.0516x; 1.0516x over previous
"""MoE gate routing kernel for Trainium2 (8 NeuronCores, data-parallel over tokens).

Computes, for x[8192,7168], weight[256,7168], bias[256]:
    scores = sigmoid(x @ weight.T + bias)            # [N, 256]
    group top-2 sums over 8 groups of 32 -> pick best group
    top-8 experts within best group (global indices), weights = renormalized
    sigmoid scores * 2.5
Returns (w [8192,8] f32, idx [8192,8] i32).

Strategy: shard tokens 8-way (1024/core). Host pre-packs x and weight into the
exact SBUF tile layouts so every DMA descriptor line is a long contiguous run
(28KB/partition vs 1KB for a naive transposed layout). Weights load as 4
K-groups so the first matmuls only wait on ~5.5MB of DMA instead of ~15MB.
Outputs accumulate in SBUF and leave as one DMA per tensor at the end, keeping
the SP DMA stream free of mid-loop waits on the vector pipeline. Matmul runs
as float32r (full-rate fp32). Bias is preloaded into PSUM via a K=1
ones-matmul. Sigmoid on ScalarE; group-top2 / top-8 / renorm on VectorE via
tensor_reduce, match_replace, max/max_index.
"""

import sys

sys.path.insert(0, "/opt/trn_rl_repo")

from concurrent.futures import ThreadPoolExecutor

import numpy as np

import concourse.bass as bass
from concourse import bacc
import concourse.mybir as mybir
from concourse.bass_utils import run_bass_kernel_spmd
from concourse.tile import TileContext

N_CORES = 8
N_TOK = 8192
TOK_PC = N_TOK // N_CORES  # 1024 tokens per core
D = 7168
E = 256
G = 8  # groups
EPG = E // G  # 32 experts per group
TOPK = 8
ROUTE_SCALE = 2.5
KC = D // 128  # 56 k-chunks
XBUF_T = 128  # tokens per x DMA buffer
NBUF = TOK_PC // XBUF_T  # 8 buffers == token tiles
NQ = 4  # weight K-groups
KCQ = KC // NQ  # 14 k-chunks per group

f32 = mybir.dt.float32
f32r = mybir.dt.float32r
i32 = mybir.dt.int32
u32 = mybir.dt.uint32
AX = mybir.AxisListType
OP = mybir.AluOpType
ACTF = mybir.ActivationFunctionType

_cache = {}
LAST_RESULTS = None


def _build():
    nc = bacc.Bacc(None, target_bir_lowering=False)

    # host-packed: x5[tb, p, c, n] = x[tok0 + tb*XBUF_T + n, c*128 + p]
    # full-buffer DMA: 28.7KB contiguous per partition; a c-slice quarter DMA
    # still gets one contiguous 7.2KB run per partition.
    x5 = nc.declare_dram_parameter("x5", [NBUF, 128, KC, XBUF_T], f32, isOutput=False)
    # w5[q, p, c, e] = weight[e, (q*KCQ+c)*128 + p]
    w5 = nc.declare_dram_parameter("w5", [NQ, 128, KCQ, E], f32, isOutput=False)
    bias = nc.declare_dram_parameter("bias", [1, E], f32, isOutput=False)
    # outputs tile-major: w_out[i, p, k] = w for token i*128+p
    w_out = nc.declare_dram_parameter("w_out", [NBUF, 128, TOPK], f32, isOutput=True)
    idx_out = nc.declare_dram_parameter("idx_out", [NBUF, 128, TOPK], i32, isOutput=True)

    with TileContext(nc) as tc:
        with (
            tc.tile_pool(name="const", bufs=1) as cpool,
            tc.tile_pool(name="xbuf", bufs=3) as xpool,
            tc.tile_pool(name="sb", bufs=3) as spool,
            tc.tile_pool(name="small", bufs=3) as mpool,
            tc.tile_pool(name="out", bufs=1) as opool,
            tc.tile_pool(name="psum", bufs=6, space="PSUM") as ppool,
        ):
            bias_sb = cpool.tile([1, E], f32)
            nc.sync.dma_start(out=bias_sb, in_=bias[:, :])
            ones_sb = cpool.tile([1, 128], f32)
            nc.vector.memset(ones_sb, 1.0)

            # weight K-groups: separate tiles so early matmuls gate per-group
            wt = [
                cpool.tile([128, KCQ, E], f32r, name=f"wt{q}") for q in range(NQ)
            ]
            nc.sync.dma_start(out=wt[0], in_=w5[0].bitcast(f32r))
            nc.sync.dma_start(out=wt[1], in_=w5[1].bitcast(f32r))

            # output accumulation tiles (free-dim slice per token tile)
            w_acc = opool.tile([128, NBUF, TOPK], f32)
            i_acc = opool.tile([128, NBUF, TOPK], u32)

            for tb in range(NBUF):
                # buffers 0..NBUF-3: one full-buffer DMA (28.7KB lines, best
                # bandwidth). Last two buffers: quarter-split so the PE's idle
                # gaps at the stream tail stay under the ~3.4us HAM
                # re-throttle window and the final tile computes at full clock.
                if tb < NBUF - 2:
                    xt = xpool.tile([128, KC, XBUF_T], f32r, tag="xt")
                    nc.sync.dma_start(out=xt, in_=x5[tb].bitcast(f32r))
                    xq = None
                else:
                    xq = []
                    for q in range(NQ):
                        xqt = xpool.tile(
                            [128, KCQ, XBUF_T], f32r, tag=f"xq{q}", bufs=1
                        )
                        nc.sync.dma_start(
                            out=xqt,
                            in_=x5[tb, :, q * KCQ : (q + 1) * KCQ, :].bitcast(f32r),
                        )
                        xq.append(xqt)
                if tb == 0:
                    nc.sync.dma_start(out=wt[2], in_=w5[2].bitcast(f32r))
                    nc.sync.dma_start(out=wt[3], in_=w5[3].bitcast(f32r))

                ps = ppool.tile([128, E], f32, tag="ps")
                # bias preload: ps[t, e] = 1 * bias[e]
                nc.tensor.matmul(
                    out=ps, lhsT=ones_sb, rhs=bias_sb, start=True, stop=False
                )
                for c in range(KC):
                    q, cq = divmod(c, KCQ)
                    nc.tensor.matmul(
                        out=ps,
                        lhsT=(
                            xt[:, c, :] if xq is None else xq[q][:, cq, :]
                        ),
                        rhs=wt[q][:, cq, :],
                        start=False,
                        stop=(c == KC - 1),
                    )

                sig = spool.tile([128, G, EPG], f32, tag="sig")
                nc.scalar.activation(
                    out=sig.rearrange("p g e -> p (g e)"), in_=ps, func=ACTF.Sigmoid
                )
                sig_flat = sig.rearrange("p g e -> p (g e)")

                # group top-2 sum
                m1 = mpool.tile([128, G], f32, tag="m1")
                nc.vector.tensor_reduce(out=m1, in_=sig, axis=AX.X, op=OP.max)
                scr = spool.tile([128, G, EPG], f32, tag="scr")
                nc.vector.match_replace(
                    out=scr.rearrange("p g e -> p (g e)"),
                    in_to_replace=m1,
                    in_values=sig_flat,
                    imm_value=-1e30,
                )
                gs = mpool.tile([128, G], f32, tag="gs")
                nc.vector.tensor_reduce(out=gs, in_=scr, axis=AX.X, op=OP.max)
                nc.vector.tensor_add(gs, gs, m1)  # m1 + m2

                # one-hot of best group -> multiplicative mask
                gmax = mpool.tile([128, 1], f32, tag="gmax")
                nc.vector.tensor_reduce(out=gmax, in_=gs, axis=AX.X, op=OP.max)
                eq = mpool.tile([128, G], f32, tag="eq")
                nc.vector.tensor_scalar(
                    eq, gs, gmax, None, op0=OP.is_ge
                )
                # masked scores: kept group unchanged (x1.0), others -> 0.0
                masked = spool.tile([128, G, EPG], f32, tag="masked")
                for g in range(G):
                    nc.vector.tensor_scalar(
                        masked[:, g, :],
                        sig[:, g, :],
                        eq[:, g : g + 1],
                        None,
                        op0=OP.mult,
                    )
                masked_flat = masked.rearrange("p g e -> p (g e)")

                vals8 = mpool.tile([128, TOPK], f32, tag="vals8")
                nc.vector.max(out=vals8, in_=masked_flat)
                nc.vector.max_index(
                    out=i_acc[:, tb, :], in_max=vals8, in_values=masked_flat
                )

                ssum = mpool.tile([128, 1], f32, tag="ssum")
                nc.vector.tensor_reduce(out=ssum, in_=vals8, axis=AX.X, op=OP.add)
                rcp = mpool.tile([128, 1], f32, tag="rcp")
                nc.vector.reciprocal(out=rcp, in_=ssum)
                nc.vector.tensor_scalar(
                    w_acc[:, tb, :], vals8, rcp, ROUTE_SCALE, op0=OP.mult, op1=OP.mult
                )

            nc.sync.dma_start(out=w_out.rearrange("i p k -> p i k"), in_=w_acc)
            nc.sync.dma_start(
                out=idx_out.rearrange("i p k -> p i k"), in_=i_acc.bitcast(i32)
            )
    nc.compile()
    return nc


def _pack_x(x, c):
    xs = x[c * TOK_PC : (c + 1) * TOK_PC]  # [1024, 7168]
    # x5[tb, p, ck, n] = xs[tb*XBUF_T + n, ck*128 + p]
    v = xs.reshape(NBUF, XBUF_T, KC, 128)  # [tb, n, ck, p]
    return np.ascontiguousarray(v.transpose(0, 3, 2, 1))


def kernel(x, weight, bias):
    global LAST_RESULTS
    x = np.ascontiguousarray(x, dtype=np.float32)
    weight = np.ascontiguousarray(weight, dtype=np.float32)
    bias = np.ascontiguousarray(bias, dtype=np.float32).reshape(1, E)

    if "nc" not in _cache:
        _cache["nc"] = _build()
    nc = _cache["nc"]

    # w5[q, p, cq, e] = weight[e, (q*KCQ+cq)*128 + p]
    w5 = np.ascontiguousarray(
        weight.reshape(E, NQ, KCQ, 128).transpose(1, 3, 2, 0)
    )

    with ThreadPoolExecutor(N_CORES) as ex:
        x_shards = list(ex.map(lambda c: _pack_x(x, c), range(N_CORES)))

    in_maps = [
        {"x5": x_shards[c], "w5": w5, "bias": bias} for c in range(N_CORES)
    ]
    LAST_RESULTS = run_bass_kernel_spmd(nc, in_maps, list(range(N_CORES)))
    res = LAST_RESULTS.results
    w = np.concatenate(
        [res[c]["w_out"].reshape(TOK_PC, TOPK) for c in range(N_CORES)], axis=0
    )
    idx = np.concatenate(
        [res[c]["idx_out"].reshape(TOK_PC, TOPK) for c in range(N_CORES)], axis=0
    )
    return w, idx.astype(np.int32)
